# revision 12
# baseline (speedup 1.0000x reference)
"""DeepseekV2 decoder layer on 8 Trainium2 NeuronCores (Bass/Tile).

Sharding: tensor-parallel over heads (2 heads/core) for attention
(q_proj/kc/vc/o_proj sliced), TP column/row split for the MLP
(gate/up columns, down rows, I padded 1368->1408 per core), compressed-KV
projection sharded over sequence + AllGather.
Collectives: AG(kv) + 4x AllReduce(attention out, per 512-seq chunk)
+ 2x ReduceScatter(mlp+residual, per 1024-seq half).

All activations live transposed [feature, token]; matmuls run in
float32r (1 cycle/row at free-dim>=256, ~1e-4 rounding).  Softmax is
computed without max-subtraction (|scores*scale| < 5 for this input
distribution), with the k-dim on partitions: the normalizer is an
accumulated ones-matmul, applied at attention-value eviction.
attn@v_c@vc_w is fused into a single matmul via u_w = v_c @ vc_w
precompute.  Residual h is folded pre-ReduceScatter as +0.125*h on
every core (exact in fp32), so no core-dependent indexing is needed.
"""
import numpy as np

import concourse.bacc as bacc
import concourse.mybir as mybir
import concourse.tile as tile
from concourse.bass_utils import run_bass_kernel_spmd
from concourse.masks import make_identity

dt = mybir.dt
AF = mybir.ActivationFunctionType
ALU = mybir.AluOpType

S, H, NH = 2048, 2048, 16
NOPE, ROPE, KVL, VH = 128, 64, 512, 128
QHD = NOPE + ROPE
I_FULL = 10944
N_CORES = 8
HPC = NH // N_CORES          # 2 heads per core
CHUNK = 512                  # attention/AR chunk (seq)
NCH = S // CHUNK             # 4
MLPW = 1024                  # mlp half width (seq)
I_SH = I_FULL // N_CORES     # 1368
I_PAD = 1408                 # zero-padded to 11*128
NIT = I_PAD // 128           # 11
HT = H // 128                # 16
ST = S // 128                # 16
SSL = S // N_CORES           # 256 seq rows per core (kv shard)
SCALE = float(QHD) ** -0.5
EPS = 1e-6
NEG = -1e30
F32, F32R = dt.float32, dt.float32r
RG = [list(range(N_CORES))]


def _r3(ap, p=128):
    """[t*p, m] dram view -> [p, t, m] k-tiled SBUF layout."""
    return ap.rearrange("(t p) m -> p t m", p=p)


def build():
    nc = bacc.Bacc("TRN2", target_bir_lowering=False, debug=False,
                   num_devices=N_CORES)
    P = nc.declare_dram_parameter
    pr = {
        "hT": P("hT", [H, S], F32, False),
        "hkv": P("hkv", [H, SSL], F32, False),
        "wqn": P("wqn", [H, HPC * NOPE], F32, False),
        "wqr": P("wqr", [H, HPC * ROPE], F32, False),
        "wkva": P("wkva", [H, KVL + ROPE], F32, False),
        "kc2": P("kc2", [NOPE, HPC * KVL], F32, False),
        "vc2": P("vc2", [KVL, HPC * VH], F32, False),
        "wo": P("wo", [HPC * VH, H], F32, False),
        "wg": P("wg", [H, I_PAD], F32, False),
        "wu": P("wu", [H, I_PAD], F32, False),
        "wd": P("wd", [I_PAD, H], F32, False),
        "cosn": P("cosn", [S, ROPE], F32, False),
        "sinn": P("sinn", [S, ROPE], F32, False),
        "cos2n": P("cos2n", [S, 2 * ROPE], F32, False),
        "sin2n": P("sin2n", [S, 2 * ROPE], F32, False),
        "ln1": P("ln1", [128, HT], F32, False),
        "ln2": P("ln2", [128, HT], F32, False),
        "lnkv": P("lnkv", [1, KVL], F32, False),
        "maskneg": P("maskneg", [128, 128], F32, False),
        "outT": P("outT", [H // N_CORES, S], F32, True),
    }
    with tile.TileContext(nc) as tc:
        _body(nc, tc, pr)
    nc.compile()
    return nc


def _body(nc, tc, pr):
    p_hT, p_out = pr["hT"], pr["outT"]

    with (
        tc.tile_pool(name="const", bufs=1) as const,
        tc.tile_pool(name="dram", bufs=1, space="DRAM") as dram,
        tc.tile_pool(name="ps", bufs=1, space="PSUM") as ps,
    ):
        # ---- constants
        ident = const.tile([128, 128], F32)
        make_identity(nc, ident)
        ones_f = const.tile([128, 1], F32)
        nc.gpsimd.memset(ones_f[:], 1.0)
        ones_r = const.tile([128, 1], F32R)
        nc.vector.tensor_copy(ones_r[:], ones_f[:])
        ln1 = const.tile([128, HT], F32)
        nc.sync.dma_start(ln1[:], pr["ln1"][:])
        ln2 = const.tile([128, HT], F32)
        nc.sync.dma_start(ln2[:], pr["ln2"][:])
        mask = const.tile([128, 128], F32)
        nc.sync.dma_start(mask[:], pr["maskneg"][:])
        eps_t = const.tile([128, 1], F32)
        nc.gpsimd.memset(eps_t[:], EPS)

        # ---- dram scratch for collectives
        kv_in = dram.tile([SSL, KVL + ROPE], F32, tag="kv_in", name="kv_in")
        kv_full = dram.tile([S, KVL + ROPE], F32, tag="kv_full", name="kv_full", addr_space="Shared")
        ar_in = [dram.tile([H, CHUNK], F32, tag=f"ar_in{c}", name=f"ar_in{c}") for c in range(NCH)]
        ar_out = [dram.tile([H, CHUNK], F32, tag=f"ar_out{c}", name=f"ar_out{c}", addr_space="Shared") for c in range(NCH)]
        rs_in = [dram.tile([H, MLPW], F32, tag=f"rs_in{m}", name=f"rs_in{m}") for m in range(2)]
        rs_out = [dram.tile([H // N_CORES, MLPW], F32, tag=f"rs_out{m}", name=f"rs_out{m}")
                  for m in range(2)]

        with tc.tile_pool(name="attn", bufs=1) as attn:
            # ---- attention-phase residents
            v_cT = attn.tile([128, 4, S], F32R, tag="v_cT")
            k_peT = attn.tile([ROPE, S], F32R, tag="k_peT")
            kpe_rp = attn.tile([ROPE, S], F32, tag="kpe_rp")  # roped, un-rms'd
            u_w = attn.tile([128, ST, HPC * VH], F32R, tag="u_w")
            wqn_s = attn.tile([128, HT, HPC * NOPE], F32R, tag="wqn_s")
            wqr_s = attn.tile([128, HT, HPC * ROPE], F32R, tag="wqr_s")
            kc_sb = attn.tile([128, HPC * KVL], F32R, tag="kc_sb")
            wo_sb = attn.tile([128, HPC, H], F32R, tag="wo_sb")
            nc.sync.dma_start(kc_sb[:], pr["kc2"][:].bitcast(F32R))
            nc.sync.dma_start(wo_sb[:], _r3(pr["wo"][:]).bitcast(F32R))

            # ======== P1: kv for this core's seq slice + AllGather ========
            with tc.tile_pool(name="prep1", bufs=1) as prep1:
                wkva_s = prep1.tile([128, HT, KVL + ROPE], F32R, tag="wkva_s")
                for t in range(HT):
                    wraw = prep1.tile([128, KVL + ROPE], F32, tag="wraw", bufs=2)
                    nc.sync.dma_start(wraw[:],
                                      pr["wkva"][t * 128:(t + 1) * 128, :])
                    nc.vector.tensor_scalar_mul(wkva_s[:, t, :], wraw[:],
                                                ln1[:, t:t + 1])
                # scaled q weights (attn residents)
                for t in range(HT):
                    wq_r = prep1.tile([128, HPC * NOPE], F32, tag="wq_r", bufs=2)
                    nc.sync.dma_start(wq_r[:],
                                      pr["wqn"][t * 128:(t + 1) * 128, :])
                    nc.vector.tensor_scalar_mul(wqn_s[:, t, :], wq_r[:],
                                                ln1[:, t:t + 1])
                    wr_r = prep1.tile([128, HPC * ROPE], F32, tag="wr_r", bufs=2)
                    nc.sync.dma_start(wr_r[:],
                                      pr["wqr"][t * 128:(t + 1) * 128, :])
                    nc.vector.tensor_scalar_mul(wqr_s[:, t, :], wr_r[:],
                                                ln1[:, t:t + 1])

                for st2 in range(SSL // 128):   # 2
                    pkv1 = ps.tile([128, KVL], F32, tag="sc", bufs=2)
                    pkv2 = ps.tile([128, ROPE], F32, tag="ev", bufs=2)
                    for t in range(HT):
                        hk = prep1.tile([128, 128], F32R, tag="hk", bufs=4)
                        nc.sync.dma_start(
                            hk[:], pr["hkv"][t * 128:(t + 1) * 128,
                                             st2 * 128:(st2 + 1) * 128]
                            .bitcast(F32R))
                        nc.tensor.matmul(pkv1[:], hk[:], wkva_s[:, t, :KVL],
                                         start=(t == 0), stop=(t == HT - 1))
                        nc.tensor.matmul(pkv2[:], hk[:], wkva_s[:, t, KVL:],
                                         start=(t == 0), stop=(t == HT - 1))
                    kvsb = prep1.tile([128, KVL + ROPE], F32, tag="kvsb", bufs=2)
                    nc.vector.tensor_copy(kvsb[:, :KVL], pkv1[:])
                    nc.vector.tensor_copy(kvsb[:, KVL:], pkv2[:])
                    nc.sync.dma_start(kv_in[st2 * 128:(st2 + 1) * 128, :],
                                      kvsb[:])
                nc.gpsimd.collective_compute(
                    "AllGather", ALU.bypass, replica_groups=RG,
                    ins=[kv_in.opt()], outs=[kv_full.opt()])

            # ======== P2: gathered kv -> v_cT, roped k_pe; u_w ========
            with tc.tile_pool(name="prep2", bufs=1) as prep2:
                lnkv1 = prep2.tile([1, KVL], F32, tag="lnkv1")
                nc.sync.dma_start(lnkv1[:], pr["lnkv"][:])
                lnkvb = prep2.tile([128, KVL], F32, tag="lnkvb")
                nc.gpsimd.partition_broadcast(lnkvb[:], lnkv1[:])
                for st in range(ST):
                    ksl = slice(st * 128, (st + 1) * 128)
                    kvt = prep2.tile([128, KVL + ROPE], F32, tag="kvt", bufs=3)
                    nc.sync.dma_start(kvt[:], kv_full[ksl, :])
                    scr = ps.tile([128, KVL], F32, tag="sc", bufs=2)
                    ssk = prep2.tile([128, 1], F32, tag="ssk", bufs=2)
                    nc.scalar.activation(scr[:], kvt[:, :KVL], AF.Square,
                                         accum_out=ssk[:])
                    srt = prep2.tile([128, 1], F32, tag="srt", bufs=2)
                    nc.scalar.activation(srt[:], ssk[:], AF.Sqrt,
                                         scale=1.0 / KVL, bias=eps_t[:])
                    invk = prep2.tile([128, 1], F32, tag="invk", bufs=2)
                    nc.vector.reciprocal(invk[:], srt[:])
                    vsc = prep2.tile([128, KVL], F32, tag="vsc", bufs=2)
                    nc.vector.tensor_scalar_mul(vsc[:], kvt[:, :KVL], invk[:])
                    nc.vector.tensor_mul(vsc[:], vsc[:], lnkvb[:])
                    for ct in range(4):
                        pt = ps.tile([128, 128], F32, tag="ev", bufs=2)
                        nc.tensor.transpose(
                            pt[:], vsc[:, ct * 128:(ct + 1) * 128], ident[:])
                        nc.vector.tensor_copy(v_cT[:, ct, ksl], pt[:])
                    # k_pe: rope in natural layout (free-dim slices), then
                    # transpose into kpe_rp columns
                    cno = prep2.tile([128, ROPE], F32, tag="cno", bufs=2)
                    sno = prep2.tile([128, ROPE], F32, tag="sno", bufs=2)
                    nc.sync.dma_start(cno[:], pr["cosn"][ksl, :])
                    nc.sync.dma_start(sno[:], pr["sinn"][ksl, :])
                    kpn = prep2.tile([128, ROPE], F32, tag="kpn", bufs=2)
                    t1 = prep2.tile([128, 32], F32, tag="t1", bufs=2)
                    t2 = prep2.tile([128, 32], F32, tag="t2", bufs=2)
                    nc.vector.tensor_mul(t1[:], kvt[:, KVL:KVL + 32],
                                         cno[:, 0:32])
                    nc.vector.tensor_mul(t2[:], kvt[:, KVL + 32:], sno[:, 0:32])
                    nc.vector.tensor_sub(kpn[:, 0:32], t1[:], t2[:])
                    nc.vector.tensor_mul(t1[:], kvt[:, KVL + 32:],
                                         cno[:, 32:64])
                    nc.vector.tensor_mul(t2[:], kvt[:, KVL:KVL + 32],
                                         sno[:, 32:64])
                    nc.vector.tensor_add(kpn[:, 32:64], t1[:], t2[:])
                    ptk = ps.tile([ROPE, 128], F32, tag="ev", bufs=2)
                    nc.tensor.transpose(ptk[:], kpn[:], ident[:])
                    nc.vector.tensor_copy(kpe_rp[:, ksl], ptk[:])

                # u_w = v_c @ vc2 : [S, 256]
                vc2_sb = prep2.tile([128, 4, HPC * VH], F32R, tag="vc2_sb")
                nc.sync.dma_start(vc2_sb[:], _r3(pr["vc2"][:]).bitcast(F32R))
                for kt in range(ST):
                    pu = ps.tile([128, HPC * VH], F32, tag="ev", bufs=2)
                    for ct in range(4):
                        nc.tensor.matmul(
                            pu[:], v_cT[:, ct, kt * 128:(kt + 1) * 128],
                            vc2_sb[:, ct, :], start=(ct == 0), stop=(ct == 3))
                    nc.vector.tensor_copy(u_w[:, kt, :], pu[:])

            # ======== P3: attention chunks ========
            with tc.tile_pool(name="ach", bufs=1) as ach:
                for c in range(NCH):
                    csl = slice(c * CHUNK, (c + 1) * CHUNK)
                    hts = []
                    for t in range(HT):
                        ht_t = ach.tile([128, CHUNK], F32R, tag="hid", bufs=16)
                        nc.sync.dma_start(
                            ht_t[:],
                            p_hT[t * 128:(t + 1) * 128, csl].bitcast(F32R))
                        hts.append(ht_t)
                    # rms1 over h (ones-matmul on squares)
                    pss = ps.tile([1, CHUNK], F32, tag="sum", bufs=1)
                    for t in range(HT):
                        sq = ach.tile([128, CHUNK], F32R, tag="sq", bufs=2)
                        nc.vector.tensor_mul(sq[:], hts[t][:].bitcast(F32),
                                             hts[t][:].bitcast(F32))
                        nc.tensor.matmul(pss[:], ones_r[:], sq[:],
                                         start=(t == 0), stop=(t == HT - 1))
                    s1 = ach.tile([1, CHUNK], F32, tag="s1", bufs=1)
                    nc.scalar.activation(s1[:], pss[:], AF.Sqrt,
                                         scale=1.0 / H, bias=eps_t[0:1, :])
                    nc.vector.reciprocal(s1[:], s1[:])
                    invb = ach.tile([128, CHUNK], F32, tag="invb", bufs=2)
                    nc.gpsimd.partition_broadcast(invb[:], s1[:])
                    # finalize k_peT columns for this chunk
                    nc.vector.tensor_mul(k_peT[:, csl], kpe_rp[:, csl],
                                         invb[0:ROPE, :])

                    # q projections (nope per head, then shared rope block)
                    q_cT, q_peT = [], []
                    for h in range(HPC):
                        pqn = ps.tile([128, CHUNK], F32, tag="ev", bufs=2)
                        for t in range(HT):
                            nc.tensor.matmul(
                                pqn[:], wqn_s[:, t, h * NOPE:(h + 1) * NOPE],
                                hts[t][:], start=(t == 0), stop=(t == HT - 1))
                        qn = ach.tile([128, CHUNK], F32R, tag=f"qn{h}", bufs=1)
                        nc.vector.tensor_mul(qn[:], pqn[:], invb[:])
                        qc = ach.tile([128, 4, CHUNK], F32R, tag=f"qc{h}",
                                      bufs=1)
                        for ct in range(4):
                            pqc = ps.tile([128, CHUNK], F32, tag="ev", bufs=2)
                            nc.tensor.matmul(
                                pqc[:],
                                kc_sb[:, h * KVL + ct * 128:
                                      h * KVL + (ct + 1) * 128],
                                qn[:], start=True, stop=True)
                            nc.vector.tensor_copy(qc[:, ct, :], pqc[:])
                        q_cT.append(qc)
                    q_pe0 = ach.tile([ROPE, CHUNK], F32R, tag="qpe0", bufs=1)
                    q_pe1 = ach.tile([ROPE, CHUNK], F32R, tag="qpe1", bufs=1)
                    q_peT = [q_pe0, q_pe1]
                    pqr = ps.tile([128, CHUNK], F32, tag="ev", bufs=2)
                    for t in range(HT):
                        nc.tensor.matmul(pqr[:], wqr_s[:, t, :], hts[t][:],
                                         start=(t == 0), stop=(t == HT - 1))
                    qrT = ach.tile([128, CHUNK], F32, tag="qrT", bufs=1)
                    nc.vector.tensor_mul(qrT[:], pqr[:], invb[:])
                    for sub in range(CHUNK // 128):
                        ptn = ps.tile([128, 128], F32, tag="ev", bufs=2)
                        nc.tensor.transpose(
                            ptn[:], qrT[:, sub * 128:(sub + 1) * 128], ident[:])
                        cosn = ach.tile([128, 2 * ROPE], F32, tag="cosn",
                                        bufs=2)
                        sinn = ach.tile([128, 2 * ROPE], F32, tag="sinn",
                                        bufs=2)
                        row0 = c * CHUNK + sub * 128
                        nc.sync.dma_start(cosn[:],
                                          pr["cos2n"][row0:row0 + 128, :])
                        nc.sync.dma_start(sinn[:],
                                          pr["sin2n"][row0:row0 + 128, :])
                        rnat = ach.tile([128, 2 * ROPE], F32, tag="rnat",
                                        bufs=2)
                        for b in (0, ROPE):
                            ta = ach.tile([128, 32], F32, tag="ta", bufs=2)
                            tb = ach.tile([128, 32], F32, tag="tb", bufs=2)
                            nc.vector.tensor_mul(ta[:], ptn[:, b:b + 32],
                                                 cosn[:, b:b + 32])
                            nc.vector.tensor_mul(tb[:], ptn[:, b + 32:b + 64],
                                                 sinn[:, b:b + 32])
                            nc.vector.tensor_sub(rnat[:, b:b + 32], ta[:],
                                                 tb[:])
                            tc2 = ach.tile([128, 32], F32, tag="tc2", bufs=2)
                            td = ach.tile([128, 32], F32, tag="td", bufs=2)
                            nc.vector.tensor_mul(tc2[:],
                                                 ptn[:, b + 32:b + 64],
                                                 cosn[:, b + 32:b + 64])
                            nc.vector.tensor_mul(td[:], ptn[:, b:b + 32],
                                                 sinn[:, b + 32:b + 64])
                            nc.vector.tensor_add(rnat[:, b + 32:b + 64],
                                                 tc2[:], td[:])
                        ptb = ps.tile([128, 128], F32, tag="ev", bufs=2)
                        nc.tensor.transpose(ptb[:], rnat[:], ident[:])
                        ssl2 = slice(sub * 128, (sub + 1) * 128)
                        nc.vector.tensor_copy(q_peT[0][:, ssl2],
                                              ptb[0:ROPE, :])
                        nc.vector.tensor_copy(q_peT[1][:, ssl2],
                                              ptb[ROPE:2 * ROPE, :])

                    # scores -> exp -> av / sum accumulate
                    avT = []
                    nkt = 4 * (c + 1)
                    for h in range(HPC):
                        pav = ps.tile([128, CHUNK], F32, tag="ev", bufs=2)
                        psm = ps.tile([1, CHUNK], F32, tag="sum", bufs=1)
                        for kt in range(nkt):
                            ksl = slice(kt * 128, (kt + 1) * 128)
                            psc = ps.tile([128, CHUNK], F32, tag="sc", bufs=2)
                            for ct in range(4):
                                nc.tensor.matmul(
                                    psc[:], v_cT[:, ct, ksl], q_cT[h][:, ct, :],
                                    start=(ct == 0), stop=False)
                            nc.tensor.matmul(psc[:], k_peT[:, ksl],
                                             q_peT[h][:], start=False,
                                             stop=True)
                            j = kt - 4 * c
                            if j >= 0:
                                if j > 0:
                                    nc.vector.memset(psc[:, 0:j * 128], NEG)
                                nc.vector.tensor_tensor(
                                    psc[:, j * 128:(j + 1) * 128],
                                    psc[:, j * 128:(j + 1) * 128],
                                    mask[:], ALU.add)
                            pT = ach.tile([128, CHUNK], F32R, tag="pT", bufs=2)
                            nc.scalar.activation(pT[:], psc[:], AF.Exp,
                                                 scale=SCALE)
                            nc.tensor.matmul(
                                pav[:], u_w[:, kt, h * VH:(h + 1) * VH], pT[:],
                                start=(kt == 0), stop=(kt == nkt - 1))
                            nc.tensor.matmul(psm[:], ones_r[:], pT[:],
                                             start=(kt == 0),
                                             stop=(kt == nkt - 1))
                        rec = ach.tile([1, CHUNK], F32, tag="rec", bufs=1)
                        nc.vector.reciprocal(rec[:], psm[:])
                        recb = ach.tile([128, CHUNK], F32, tag="recb", bufs=2)
                        nc.gpsimd.partition_broadcast(recb[:], rec[:])
                        av = ach.tile([128, CHUNK], F32R, tag=f"av{h}", bufs=1)
                        nc.vector.tensor_mul(av[:], pav[:], recb[:])
                        avT.append(av)

                    # o projection partial -> AR input
                    for jt in range(HT):
                        po = ps.tile([128, CHUNK], F32, tag="ev", bufs=2)
                        for h in range(HPC):
                            nc.tensor.matmul(
                                po[:], wo_sb[:, h, jt * 128:(jt + 1) * 128],
                                avT[h][:], start=(h == 0), stop=(h == HPC - 1))
                        osb = ach.tile([128, CHUNK], F32, tag="osb", bufs=2)
                        nc.vector.tensor_copy(osb[:], po[:])
                        nc.sync.dma_start(
                            ar_in[c][jt * 128:(jt + 1) * 128, :], osb[:])
                    nc.gpsimd.collective_compute(
                        "AllReduce", ALU.add, replica_groups=RG,
                        ins=[ar_in[c].opt()], outs=[ar_out[c].opt()])

        # ======== P4: MLP halves ========
        with tc.tile_pool(name="mlp", bufs=1) as mlp:
            for mc in range(2):
                x2 = mlp.tile([128, HT, MLPW], F32R, tag="x2", bufs=1)
                inv2b = []
                for sub in range(2):
                    c = 2 * mc + sub
                    csl = slice(c * CHUNK, (c + 1) * CHUNK)
                    pss2 = ps.tile([1, CHUNK], F32, tag="sum", bufs=1)
                    for t in range(HT):
                        tar = mlp.tile([128, CHUNK], F32, tag="tar", bufs=2)
                        nc.sync.dma_start(
                            tar[:], ar_out[c][t * 128:(t + 1) * 128, :])
                        th = mlp.tile([128, CHUNK], F32, tag="th", bufs=2)
                        nc.sync.dma_start(th[:],
                                          p_hT[t * 128:(t + 1) * 128, csl])
                        tsum = mlp.tile([128, CHUNK], F32, tag="tsum", bufs=2)
                        nc.vector.tensor_add(tsum[:], th[:], tar[:])
                        nc.vector.tensor_scalar_mul(
                            x2[:, t, sub * CHUNK:(sub + 1) * CHUNK], tsum[:],
                            ln2[:, t:t + 1])
                        sq2 = mlp.tile([128, CHUNK], F32R, tag="sq2", bufs=2)
                        nc.vector.tensor_mul(sq2[:], tsum[:], tsum[:])
                        nc.tensor.matmul(pss2[:], ones_r[:], sq2[:],
                                         start=(t == 0), stop=(t == HT - 1))
                    s2 = mlp.tile([1, CHUNK], F32, tag="s2", bufs=2)
                    nc.scalar.activation(s2[:], pss2[:], AF.Sqrt,
                                         scale=1.0 / H, bias=eps_t[0:1, :])
                    iv2 = mlp.tile([1, CHUNK], F32, tag="iv2", bufs=2)
                    nc.vector.reciprocal(iv2[:], s2[:])
                    ivb = mlp.tile([128, CHUNK], F32, tag=f"ivb{sub}", bufs=1)
                    nc.gpsimd.partition_broadcast(ivb[:], iv2[:])
                    inv2b.append(ivb)

                aT = mlp.tile([128, NIT, MLPW], F32R, tag="aT", bufs=1)
                for it in range(NIT):
                    wg_t = mlp.tile([128, HT, 128], F32R, tag="wg_t", bufs=2)
                    nc.sync.dma_start(
                        wg_t[:],
                        _r3(pr["wg"][:, it * 128:(it + 1) * 128]).bitcast(F32R))
                    wu_t = mlp.tile([128, HT, 128], F32R, tag="wu_t", bufs=2)
                    nc.sync.dma_start(
                        wu_t[:],
                        _r3(pr["wu"][:, it * 128:(it + 1) * 128]).bitcast(F32R))
                    for sub in range(2):
                        xsl = slice(sub * CHUNK, (sub + 1) * CHUNK)
                        pg = ps.tile([128, CHUNK], F32, tag="gu", bufs=3)
                        for t in range(HT):
                            nc.tensor.matmul(pg[:], wg_t[:, t, :],
                                             x2[:, t, xsl], start=(t == 0),
                                             stop=(t == HT - 1))
                        pu2 = ps.tile([128, CHUNK], F32, tag="gu", bufs=3)
                        for t in range(HT):
                            nc.tensor.matmul(pu2[:], wu_t[:, t, :],
                                             x2[:, t, xsl], start=(t == 0),
                                             stop=(t == HT - 1))
                        nc.vector.tensor_mul(pg[:], pg[:], inv2b[sub][:])
                        nc.vector.tensor_mul(pu2[:], pu2[:], inv2b[sub][:])
                        gsb = mlp.tile([128, CHUNK], F32, tag="gsb", bufs=2)
                        nc.scalar.activation(gsb[:], pg[:], AF.Silu)
                        nc.vector.tensor_tensor(aT[:, it, xsl], gsb[:],
                                                pu2[:], ALU.mult)

                for jt in range(HT):
                    wd_t = mlp.tile([128, NIT, 128], F32R, tag="wd_t", bufs=2)
                    nc.sync.dma_start(
                        wd_t[:],
                        _r3(pr["wd"][:, jt * 128:(jt + 1) * 128]).bitcast(F32R))
                    for sub in range(2):
                        c = 2 * mc + sub
                        csl = slice(c * CHUNK, (c + 1) * CHUNK)
                        pd = ps.tile([128, CHUNK], F32, tag="ev", bufs=2)
                        for it in range(NIT):
                            nc.tensor.matmul(
                                pd[:], wd_t[:, it, :],
                                aT[:, it, sub * CHUNK:(sub + 1) * CHUNK],
                                start=(it == 0), stop=(it == NIT - 1))
                        # + 0.125*h (residual; exact across the 8-way RS sum)
                        th2 = mlp.tile([128, CHUNK], F32, tag="th2", bufs=2)
                        nc.sync.dma_start(th2[:],
                                          p_hT[jt * 128:(jt + 1) * 128, csl])
                        ta2 = mlp.tile([128, CHUNK], F32, tag="ta2", bufs=2)
                        nc.sync.dma_start(
                            ta2[:], ar_out[c][jt * 128:(jt + 1) * 128, :])
                        hsum = mlp.tile([128, CHUNK], F32, tag="hsum", bufs=2)
                        nc.vector.tensor_add(hsum[:], th2[:], ta2[:])
                        nc.vector.tensor_scalar_mul(hsum[:], hsum[:], 0.125)
                        dsb = mlp.tile([128, CHUNK], F32, tag="dsb", bufs=2)
                        nc.vector.tensor_tensor(dsb[:], pd[:], hsum[:],
                                                ALU.add)
                        nc.sync.dma_start(
                            rs_in[mc][jt * 128:(jt + 1) * 128,
                                      sub * CHUNK:(sub + 1) * CHUNK], dsb[:])
                nc.gpsimd.collective_compute(
                    "ReduceScatter", ALU.add, replica_groups=RG,
                    ins=[rs_in[mc].opt()], outs=[rs_out[mc].opt()])
                nc.sync.dma_start(p_out[:, mc * MLPW:(mc + 1) * MLPW],
                                  rs_out[mc][:])


_NC_CACHE = None


def _get_nc():
    global _NC_CACHE
    if _NC_CACHE is None:
        _NC_CACHE = build()
    return _NC_CACHE


def make_in_maps(inputs):
    f = np.float32
    hid = np.asarray(inputs["hidden_states"], f)[0]          # [S, H]
    hT = np.ascontiguousarray(hid.T)
    cos = np.asarray(inputs["cos"], f)
    sin = np.asarray(inputs["sin"], f)
    qw = np.asarray(inputs["q_proj_w"], f)
    kcw = np.asarray(inputs["kc_w"], f)
    vcw = np.asarray(inputs["vc_w"], f)
    ow = np.asarray(inputs["o_proj_w"], f)
    gup = np.asarray(inputs["gate_up_w"], f)
    dw = np.asarray(inputs["down_w"], f)
    common = {
        "hT": hT,
        "wkva": np.ascontiguousarray(np.asarray(inputs["kv_a_w"], f)),
        "cosn": np.ascontiguousarray(cos),
        "sinn": np.ascontiguousarray(sin),
        "cos2n": np.ascontiguousarray(np.concatenate([cos, cos], 1)),
        "sin2n": np.ascontiguousarray(np.concatenate([sin, sin], 1)),
        "ln1": np.ascontiguousarray(
            np.asarray(inputs["ln1_w"], f).reshape(HT, 128).T),
        "ln2": np.ascontiguousarray(
            np.asarray(inputs["ln2_w"], f).reshape(HT, 128).T),
        "lnkv": np.ascontiguousarray(
            np.asarray(inputs["kv_a_ln_w"], f).reshape(1, KVL)),
        "maskneg": np.ascontiguousarray(np.where(
            np.arange(128)[None, :] >= np.arange(128)[:, None],
            0.0, NEG)).astype(f),
    }
    in_maps = []
    for c in range(N_CORES):
        h0 = HPC * c
        m = dict(common)
        m["hkv"] = np.ascontiguousarray(hT[:, c * SSL:(c + 1) * SSL])
        m["wqn"] = np.ascontiguousarray(np.concatenate(
            [qw[:, h * QHD:h * QHD + NOPE] for h in range(h0, h0 + HPC)], 1))
        m["wqr"] = np.ascontiguousarray(np.concatenate(
            [qw[:, h * QHD + NOPE:(h + 1) * QHD] for h in range(h0, h0 + HPC)],
            1))
        m["kc2"] = np.ascontiguousarray(np.concatenate(
            [kcw[h] for h in range(h0, h0 + HPC)], 1))
        m["vc2"] = np.ascontiguousarray(np.concatenate(
            [vcw[h] for h in range(h0, h0 + HPC)], 1))
        m["wo"] = np.ascontiguousarray(ow[h0 * VH:(h0 + HPC) * VH, :])
        m["wg"] = np.ascontiguousarray(np.pad(
            gup[:, c * I_SH:(c + 1) * I_SH], ((0, 0), (0, I_PAD - I_SH))))
        m["wu"] = np.ascontiguousarray(np.pad(
            gup[:, I_FULL + c * I_SH:I_FULL + (c + 1) * I_SH],
            ((0, 0), (0, I_PAD - I_SH))))
        m["wd"] = np.ascontiguousarray(np.pad(
            dw[c * I_SH:(c + 1) * I_SH, :], ((0, I_PAD - I_SH), (0, 0))))
        in_maps.append(m)
    return in_maps


def assemble_output(results):
    outT = np.concatenate([results[c]["outT"] for c in range(N_CORES)], 0)
    return np.ascontiguousarray(outT.T).reshape(1, S, H).astype(np.float32)


def kernel(**inputs) -> np.ndarray:
    nc = _get_nc()
    in_maps = make_in_maps(inputs)
    res = run_bass_kernel_spmd(nc, in_maps, list(range(N_CORES)))
    return assemble_output(res.results)
